# revision 23
# baseline (speedup 1.0000x reference)
"""GAT layer (LayerNorm -> QKV -> full 8-head attention with leaky_relu-before-
softmax -> out-proj -> skip) on 8 Trainium2 NeuronCores.

Sharding: (head-pair, q-half).  Core c handles heads (2f, 2f+1) with f = c % 4
and query rows [h*2048, (h+1)*2048) with h = c // 4 (the host rolls x so each
core's q rows sit at [0, 2048)).  Each core projects k/v for only its two
heads over all 4096 nodes and returns the *partial* fc output for its q-half
(the contribution of its 2 heads).  The host sums the 4 partials per q-half
and adds the skip connection + fc bias -- a pure linear unshard of the
partial-sum sharding.

Per-core pipeline:
  prologue: stream x (bf16), LayerNorm stats (bn_stats/bn_aggr ->
            rstd = exp(-0.5*ln(var+eps)) so only the natural_log_exp table
            set is ever loaded), normalize (bf16), PE-transpose to xT for the
            q-half, project qT and kT/v for kv banks 0-1.
  bank loop (4 q-banks of 512 q) x 32 kv chunks, through a manually managed
  6-bank PSUM score ring (depth 3 chunks):
      scores: two K=64 matmuls row-tiled into PE halves (tile_position (0,0)
              / (64,0), run concurrently) -> ring slot [128 kv, 2, 512] f32.
      leaky:  t = 4*relu(s) (DVE tensor_scalar for most chunks, ACT for some
              to balance the engines) -> PE identity-accumulate
              (m = s + 4*relu(s) = 5*leaky(s)) -> ACT exp(0.2*m) from PSUM,
              batched over chunk pairs (FD=2048) when the ring wraps allow.
      AV:     p.T @ [v|1] per head, M=65 (softmax denominator rides the
              matmul), accumulated over the 32 chunks in PSUM.
    per bank: softmax divide (reciprocal in [128, 8] partition-major layout
    via a DRAM bounce, broadcast back), fc partial (heads stacked K=128),
    DMA out.  fc and the JIT projections borrow ring slots for their PSUM.
  LN/transpose/projection for kv banks 2-7 are emitted just-in-time inside
  bank 0 so the PE/DVE fill the ACT-bound steady state.
"""

import sys

for _p in ("/opt/trn_rl_repo",):
    if _p not in sys.path:
        sys.path.insert(0, _p)

import numpy as np
import ml_dtypes

B, D, H, DH = 4096, 512, 8, 64
P = 128
NCORES = 8
NPAIRS = 4
QH = B // 2
NT = B // P                 # 32 kv chunks
KC = D // P                 # 4 contraction chunks
NB = B // 512               # 8 node banks
QB = QH // 512              # 4 q banks per core
NG = B // (4 * P)           # 8 LN groups of 4 node tiles
NEG_SLOPE = 0.2
LN_EPS = 1e-5
TEMP = float(np.sqrt(D))

BF16 = ml_dtypes.bfloat16

_PROGRAM = None


def _build_program(has_qb, has_kb, has_vb):
    from contextlib import ExitStack

    import concourse.bass as bass
    import concourse.bacc as bacc
    import concourse.tile as tile
    import concourse.mybir as mybir

    dt = mybir.dt
    AF = mybir.ActivationFunctionType
    OP = mybir.AluOpType

    nc = bacc.Bacc("TRN2", target_bir_lowering=False, debug=False)

    x_d = nc.dram_tensor("x", [B, D], dt.bfloat16, kind="ExternalInput").ap()
    wqT_d = nc.dram_tensor("wqT", [D, P], dt.bfloat16, kind="ExternalInput").ap()
    wkT_d = nc.dram_tensor("wkT", [D, P], dt.bfloat16, kind="ExternalInput").ap()
    wvT_d = nc.dram_tensor("wvT", [D, P], dt.bfloat16, kind="ExternalInput").ap()
    fwT_d = nc.dram_tensor("fwT", [P, D], dt.bfloat16, kind="ExternalInput").ap()
    ident_d = nc.dram_tensor("ident", [P, P], dt.bfloat16, kind="ExternalInput").ap()
    bq_d = bk_d = bvr_d = None
    if has_qb:
        bq_d = nc.dram_tensor("bq", [P], dt.float32, kind="ExternalInput").ap()
    if has_kb:
        bk_d = nc.dram_tensor("bk", [P], dt.float32, kind="ExternalInput").ap()
    if has_vb:
        bvr_d = nc.dram_tensor("bvr", [1, P], dt.bfloat16, kind="ExternalInput").ap()
    out_d = nc.dram_tensor("out", [QH, D], dt.float32, kind="ExternalOutput").ap()
    # softmax denominator bounce buffers (flat, [bank*1024 + head*512 + q])
    dden_d = nc.dram_tensor("dden", [QH * 2], dt.bfloat16).ap()
    drec_d = nc.dram_tensor("drec", [QH * 2], dt.bfloat16).ap()

    with tile.TileContext(nc) as tc, ExitStack() as ctx:
        consts = ctx.enter_context(tc.tile_pool(name="consts", bufs=1))
        persist = ctx.enter_context(tc.tile_pool(name="persist", bufs=1))
        psp = ctx.enter_context(tc.tile_pool(name="psring", bufs=1, space="PSUM"))

        ident_t = consts.tile([P, P], dt.bfloat16, name="ident_t", tag="ident")
        nc.sync.dma_start(out=ident_t[:], in_=ident_d)
        eps_t = consts.tile([P, 1], dt.float32, name="eps_t", tag="eps")
        nc.vector.memset(eps_t[:], LN_EPS)
        wq_t = [consts.tile([P, P], dt.bfloat16, name=f"wq{k}", tag=f"wq{k}") for k in range(KC)]
        wk_t = [consts.tile([P, P], dt.bfloat16, name=f"wk{k}", tag=f"wk{k}") for k in range(KC)]
        wv_t = [consts.tile([P, P], dt.bfloat16, name=f"wv{k}", tag=f"wv{k}") for k in range(KC)]
        fw_t = consts.tile([P, D], dt.bfloat16, name="fw", tag="fw")
        bq_t = bk_t = bvr_t = ones1_t = None
        if has_qb:
            bq_t = consts.tile([P, 1], dt.float32, name="bq_t", tag="bq")
        if has_kb:
            bk_t = consts.tile([P, 1], dt.float32, name="bk_t", tag="bk")
        if has_vb:
            bvr_t = consts.tile([1, P], dt.bfloat16, name="bvr_t", tag="bvr")
            ones1_t = consts.tile([1, P], dt.bfloat16, name="ones1_t", tag="ones1")
            nc.vector.memset(ones1_t[:], 1.0)

        def emit_weight_dmas():
            for k in range(KC):
                nc.sync.dma_start(out=wq_t[k][:], in_=wqT_d[k * P:(k + 1) * P, :])
                nc.sync.dma_start(out=wk_t[k][:], in_=wkT_d[k * P:(k + 1) * P, :])
                nc.sync.dma_start(out=wv_t[k][:], in_=wvT_d[k * P:(k + 1) * P, :])
            nc.sync.dma_start(out=fw_t[:], in_=fwT_d)
            if has_qb:
                nc.sync.dma_start(out=bq_t[:, 0], in_=bq_d)
            if has_kb:
                nc.sync.dma_start(out=bk_t[:, 0], in_=bk_d)
            if has_vb:
                nc.sync.dma_start(out=bvr_t[:], in_=bvr_d)

        # ---- persistent tensors ----
        xT = persist.tile([P, KC, B], dt.bfloat16, name="xT", tag="xT")
        kT = persist.tile([P, B], dt.bfloat16, name="kT", tag="kT")
        qT = persist.tile([P, QH], dt.bfloat16, name="qT", tag="qT")
        # vA[:, c, :]: [128 kv, 130]; 0:64 head0 v, 64 = 1, 65:129 head1 v,
        # 129 = 1
        vA = persist.tile([P, NT, 2 * DH + 2], dt.bfloat16, name="vA", tag="vA")
        # both heads' attention outputs stacked: rows 0:64 head0, 64:128 head1
        aT_t = persist.tile([P, QH], dt.bfloat16, name="aT", tag="aT")
        for j in range(2):
            col = DH + j * (DH + 1)
            ap = vA[:]
            ones_dst = bass.AP(tensor=ap.tensor, offset=ap.offset + col,
                               ap=[list(ap.ap[0]), [2 * DH + 2, NT], [1, 1]])
            nc.gpsimd.memset(ones_dst, 1.0)

        # LN stats: mv_t[:, 4g+j, 0] = mean, [..., 1] = var
        mv_t = persist.tile([P, NG * 4, 2], dt.float32, name="mv", tag="mv")
        rstd_t = persist.tile([P, NG * 4], dt.float32, name="rstd", tag="rstd")
        lnv_t = persist.tile([P, NG * 4], dt.float32, name="lnv", tag="lnv")

        # 6-bank PSUM score ring; chunk c occupies [:, (2c)%6:(2c)%6+2, :]
        ring = psp.tile([P, 6, 512], dt.float32, name="ring", tag="ring")

        with tc.tile_pool(name="xin", bufs=5) as xpool, \
             tc.tile_pool(name="stats", bufs=8) as spool, \
             tc.tile_pool(name="xh", bufs=3) as hpool, \
             tc.tile_pool(name="aug_ps", bufs=2, space="PSUM") as augps, \
             tc.tile_pool(name="tt", bufs=3) as tpool, \
             tc.tile_pool(name="pt", bufs=2) as ptpool, \
             tc.tile_pool(name="div", bufs=4) as dpool, \
             tc.tile_pool(name="ot", bufs=2) as opool:

            xg_t = [None] * NG

            def emit_stats(g):
                xg = xpool.tile([P, 4, D], dt.bfloat16, tag="xg", name="xg")
                src = bass.AP(tensor=x_d.tensor, offset=x_d.offset + 4 * g * P * D,
                              ap=[[D, P], [P * D, 4], [1, D]])
                nc.sync.dma_start(out=xg[:], in_=src)
                xg_t[g] = xg
                for j in range(4):
                    st6 = spool.tile([P, 6], dt.float32, tag="st6", name="st6")
                    nc.vector.bn_stats(st6[:], xg[:, j, :])
                    nc.vector.bn_aggr(mv_t[:, 4 * g + j, :], st6[:])

            def emit_rstd(g0, g1):
                """rstd = exp(-0.5*ln(var+eps)) for groups [g0, g1)."""
                s = slice(4 * g0, 4 * g1)
                nc.scalar.activation(lnv_t[:, s], mv_t[:, s, 1], AF.Ln,
                                     bias=eps_t[:, 0:1])
                nc.scalar.activation(rstd_t[:, s], lnv_t[:, s], AF.Exp,
                                     scale=-0.5)

            # helpers handing out ring banks (0..5) to borrowers
            def rbank(i):
                return ring[:, i % 6, :]

            def emit_norm_xpose(g, ri):
                xg = xg_t[g]
                for j in range(4):
                    xh = hpool.tile([P, D], dt.bfloat16, tag="xh", name="xh")
                    nc.vector.tensor_scalar(
                        out=xh[:], in0=xg[:, j, :],
                        scalar1=mv_t[:, 4 * g + j, 0:1],
                        scalar2=rstd_t[:, 4 * g + j:4 * g + j + 1],
                        op0=OP.subtract, op1=OP.mult)
                    tp = rbank(ri + j % 2).bitcast(dt.bfloat16)  # [128, 1024] bf16
                    for f in range(KC):
                        nc.tensor.transpose(
                            tp[:, f * P:(f + 1) * P],
                            xh[:, f * P:(f + 1) * P],
                            ident_t[:],
                        )
                    n0 = (4 * g + j) * P
                    xap = xT[:]
                    dst = bass.AP(tensor=xap.tensor, offset=xap.offset + n0,
                                  ap=[list(xap.ap[0]), [B, KC], [1, P]])
                    nc.vector.tensor_copy(out=dst, in_=tp[:, 0:D])

            def emit_kproj(nb, ri):
                kp = rbank(ri)
                for k in range(KC):
                    nc.tensor.matmul(
                        kp, lhsT=wk_t[k][:],
                        rhs=xT[:, k, nb * 512:(nb + 1) * 512],
                        start=(k == 0), stop=(k == KC - 1),
                        skip_group_check=True)
                if has_kb:
                    nc.vector.tensor_scalar(
                        out=kT[:, nb * 512:(nb + 1) * 512], in0=kp,
                        scalar1=bk_t[:, 0:1], scalar2=0.0,
                        op0=OP.add, op1=OP.bypass)
                else:
                    nc.vector.tensor_copy(out=kT[:, nb * 512:(nb + 1) * 512],
                                          in_=kp)

            def emit_vproj(nb, ri):
                vp = rbank(ri)
                for blk in range(4):
                    c = nb * 4 + blk
                    for k in range(KC):
                        nc.tensor.matmul(
                            vp[:, blk * P:(blk + 1) * P],
                            lhsT=xT[:, k, c * P:(c + 1) * P],
                            rhs=wv_t[k][:],
                            start=(k == 0), stop=(k == KC - 1 and not has_vb),
                            skip_group_check=True)
                    if has_vb:
                        nc.tensor.matmul(
                            vp[:, blk * P:(blk + 1) * P],
                            lhsT=ones1_t[0:1, :], rhs=bvr_t[0:1, :],
                            start=False, stop=True, skip_group_check=True)
                # one cast per node bank: ring [128, 4*128] -> vA[:, 4nb:4nb+4,
                # {0:64, 65:129}]
                ap = vA[:]
                dst = bass.AP(
                    tensor=ap.tensor, offset=ap.offset + nb * 4 * (2 * DH + 2),
                    ap=[list(ap.ap[0]), [2 * DH + 2, 4], [DH + 1, 2], [1, DH]])
                rap = ring[:]
                src = bass.AP(
                    tensor=rap.tensor, offset=rap.offset + (ri % 6) * 512,
                    ap=[list(rap.ap[0]), [P, 4], [DH, 2], [1, DH]])
                nc.vector.tensor_copy(out=dst, in_=src)

            def emit_qproj(qb, ri):
                qp = rbank(ri)
                for k in range(KC):
                    nc.tensor.matmul(
                        qp, lhsT=wq_t[k][:],
                        rhs=xT[:, k, qb * 512:(qb + 1) * 512],
                        start=(k == 0), stop=(k == KC - 1),
                        skip_group_check=True)
                if has_qb:
                    nc.vector.tensor_scalar(
                        out=qT[:, qb * 512:(qb + 1) * 512], in0=qp,
                        scalar1=bq_t[:, 0:1], scalar2=0.0,
                        op0=OP.add, op1=OP.bypass)
                else:
                    nc.vector.tensor_copy(out=qT[:, qb * 512:(qb + 1) * 512],
                                          in_=qp)

            # ---------- prologue: q-half (groups 0-3) + kv banks 0-1 ----------
            emit_stats(0)
            emit_weight_dmas()
            for g in range(1, 4):
                emit_stats(g)
            emit_rstd(0, 4)
            ri = 0
            for g in range(4):
                emit_norm_xpose(g, ri)
                ri += 2
            for qb in range(QB):
                emit_qproj(qb, ri)
                ri += 1
            for nb in range(2):
                emit_kproj(nb, ri)
                ri += 1
                emit_vproj(nb, ri)
                ri += 1

            # ---------- attention ----------
            GR = 4

            def fc_bank(qb, ri):
                for blk in range(4):
                    q0 = qb * 512 + blk * P
                    fp = rbank(ri + blk)
                    nc.tensor.matmul(fp, lhsT=aT_t[:, q0:q0 + P],
                                     rhs=fw_t[:], start=True, stop=True,
                                     skip_group_check=True)
                    ot = opool.tile([P, D], dt.float32, tag="ot", name="ot")
                    if blk % 2 == 0:
                        nc.vector.tensor_copy(out=ot[:], in_=fp)
                    else:
                        nc.scalar.copy(ot[:], fp)
                    nc.sync.dma_start(out=out_d[q0:q0 + P, :], in_=ot[:])

            # JIT work inside bank 0, keyed by chunk index.  Ring banks for
            # the borrowed PSUM are the ones this chunk is about to free
            # (rj(c)+4 is the oldest slot pair in the depth-3 ring).
            jit = {0: lambda c: emit_stats(4),
                   2: lambda c: emit_stats(5),
                   3: lambda c: (emit_kproj(2, 4), emit_vproj(2, 5)),
                   4: lambda c: emit_stats(6),
                   6: lambda c: (emit_stats(7), emit_rstd(4, 8)),
                   8: lambda c: (emit_kproj(3, 2), emit_vproj(3, 3)),
                   10: lambda c: emit_norm_xpose(4, 0),
                   13: lambda c: (emit_kproj(4, 0), emit_vproj(4, 1)),
                   16: lambda c: emit_norm_xpose(5, 2),
                   18: lambda c: (emit_kproj(5, 4), emit_vproj(5, 5)),
                   20: lambda c: emit_norm_xpose(6, 2),
                   22: lambda c: (emit_kproj(6, 0), emit_vproj(6, 1)),
                   24: lambda c: emit_norm_xpose(7, 2),
                   26: lambda c: (emit_kproj(7, 4), emit_vproj(7, 5))}

            for qb in range(QB):
                augA = augps.tile([DH + 1, 512], dt.float32, tag="aug")
                augB = augps.tile([DH + 1, 512], dt.float32, tag="aug")
                pt_g = None
                pend = []      # chunks whose exp hasn't been emitted yet

                def flush_exp(chunks):
                    """exp over the pending chunks' ring slots into pt_g."""
                    if not chunks:
                        return
                    # batch pairs whose ring slots are contiguous
                    i = 0
                    while i < len(chunks):
                        cc, rj = chunks[i]
                        if (i + 1 < len(chunks)
                                and chunks[i + 1][1] == rj + 2):
                            src = ring[:, rj:rj + 4, :]
                            dst = pt_g[:, (cc % GR) * 1024:(cc % GR) * 1024 + 2048]
                            nc.scalar.activation(dst, src, AF.Exp,
                                                 scale=NEG_SLOPE)
                            i += 2
                        else:
                            src = ring[:, rj:rj + 2, :]
                            dst = pt_g[:, (cc % GR) * 1024:(cc % GR + 1) * 1024]
                            nc.scalar.activation(dst, src, AF.Exp,
                                                 scale=NEG_SLOPE)
                            i += 1

                def emit_av(cc):
                    r = cc % GR
                    nc.tensor.matmul(
                        augA[:], lhsT=vA[:, cc, 0:DH + 1],
                        rhs=pt_g[:, r * 1024:r * 1024 + 512],
                        start=(cc == 0), stop=(cc == NT - 1))
                    nc.tensor.matmul(
                        augB[:], lhsT=vA[:, cc, DH + 1:2 * DH + 2],
                        rhs=pt_g[:, r * 1024 + 512:(r + 1) * 1024],
                        start=(cc == 0), stop=(cc == NT - 1))

                for c in range(NT):
                    if qb == 0 and c in jit:
                        jit[c](c)
                    # relu on ACT for some chunks to balance DVE/ACT load
                    act_path = (c % 4 == 3) if qb == 0 else (c % 8 == 7)
                    if c % GR == 0:
                        pt_g = ptpool.tile([P, GR * 1024], dt.bfloat16, tag="pt",
                                           name="pt")
                    rj = (2 * c) % 6
                    spc = ring[:, rj:rj + 2, :]
                    nc.tensor.matmul(
                        spc[:, 0, :],
                        lhsT=kT[0:DH, c * P:(c + 1) * P],
                        rhs=qT[0:DH, qb * 512:(qb + 1) * 512],
                        start=True, stop=True, tile_position=(0, 0),
                        skip_group_check=True)
                    nc.tensor.matmul(
                        spc[:, 1, :],
                        lhsT=kT[DH:2 * DH, c * P:(c + 1) * P],
                        rhs=qT[DH:2 * DH, qb * 512:(qb + 1) * 512],
                        start=True, stop=True, tile_position=(64, 0),
                        skip_group_check=True)
                    tt = tpool.tile([P, 1024], dt.bfloat16, tag="tt", name="tt")
                    if act_path:
                        nc.scalar.activation(tt[:], spc, AF.Relu, scale=4.0)
                    else:
                        nc.vector.tensor_scalar(
                            out=tt[:], in0=spc, scalar1=0.0, scalar2=4.0,
                            op0=OP.max, op1=OP.mult)
                    nc.tensor.matmul(
                        spc[:, 0, :], lhsT=ident_t[:], rhs=tt[:, 0:512],
                        start=False, stop=True, skip_group_check=True)
                    nc.tensor.matmul(
                        spc[:, 1, :], lhsT=ident_t[:], rhs=tt[:, 512:1024],
                        start=False, stop=True, skip_group_check=True)
                    pend.append((c, rj))
                    if len(pend) == 2 or c == NT - 1:
                        flush_exp(pend)
                        for (cc, _) in pend:
                            emit_av(cc)
                        pend = []

                # ---- softmax divide ----
                aug_sb = dpool.tile([DH + 1, 2, 512], dt.bfloat16, tag="augsb",
                                    name="augsb")
                nc.vector.tensor_copy(out=aug_sb[:, 0, :], in_=augA[:])
                nc.vector.tensor_copy(out=aug_sb[:, 1, :], in_=augB[:])
                for j in range(2):
                    nc.sync.dma_start(
                        out=dden_d[qb * 1024 + j * 512:qb * 1024 + (j + 1) * 512],
                        in_=aug_sb[DH:DH + 1, j, :])
                den8 = dpool.tile([P, 8], dt.bfloat16, tag="den8", name="den8")
                src = dden_d[qb * 1024:(qb + 1) * 1024]
                nc.sync.dma_start(
                    out=den8[:],
                    in_=bass.AP(tensor=src.tensor, offset=src.offset,
                                ap=[[8, P], [1, 8]]))
                rec8 = dpool.tile([P, 8], dt.bfloat16, tag="rec8", name="rec8")
                with nc.allow_low_precision(reason="softmax 1/den in bf16 is fine at 2e-2 tol"):
                    nc.vector.reciprocal(rec8[:], den8[:])
                dst = drec_d[qb * 1024:(qb + 1) * 1024]
                nc.sync.dma_start(
                    out=bass.AP(tensor=dst.tensor, offset=dst.offset,
                                ap=[[8, P], [1, 8]]),
                    in_=rec8[:])
                for j in range(2):
                    rb = dpool.tile([DH, 512], dt.bfloat16, tag="rb", name="rb")
                    src = drec_d[qb * 1024 + j * 512:qb * 1024 + (j + 1) * 512]
                    bcast = bass.AP(tensor=src.tensor, offset=src.offset,
                                    ap=[[0, DH], [1, 512]])
                    nc.sync.dma_start(out=rb[:], in_=bcast)
                    nc.vector.tensor_mul(
                        out=aT_t[j * DH:(j + 1) * DH, qb * 512:(qb + 1) * 512],
                        in0=aug_sb[0:DH, j, :], in1=rb[:])
                fc_bank(qb, 4)

    nc.compile()
    return nc


def _prep_inputs(in_feats, wq, wk, wv, fc_w, fc_b, ln_w, ln_b):
    ln_w = ln_w.astype(np.float32)
    ln_b = ln_b.astype(np.float32)
    wq_f = (wq.astype(np.float32) * ln_w[None, :]) / TEMP
    wk_f = wk.astype(np.float32) * ln_w[None, :]
    wv_f = wv.astype(np.float32) * ln_w[None, :]
    bq = (wq.astype(np.float32) @ ln_b) / TEMP
    bk = wk.astype(np.float32) @ ln_b
    bv = wv.astype(np.float32) @ ln_b
    has_qb = bool(np.any(bq != 0))
    has_kb = bool(np.any(bk != 0))
    has_vb = bool(np.any(bv != 0))
    x_bf = np.ascontiguousarray(in_feats.astype(np.float32)).astype(BF16)
    wqT = np.ascontiguousarray(wq_f.T).astype(BF16)
    wkT = np.ascontiguousarray(wk_f.T).astype(BF16)
    wvT = np.ascontiguousarray(wv_f.T).astype(BF16)
    fwT = np.ascontiguousarray(fc_w.astype(np.float32).T).astype(BF16)
    ident = np.eye(P, dtype=np.float32).astype(BF16)
    flags = (has_qb, has_kb, has_vb)
    x_halves = [x_bf, np.ascontiguousarray(np.roll(x_bf, -QH, axis=0))]
    in_maps = []
    for c in range(NCORES):
        f = c % NPAIRS
        h = c // NPAIRS
        m = {
            "x": x_halves[h],
            "wqT": np.ascontiguousarray(wqT[:, f * P:(f + 1) * P]),
            "wkT": np.ascontiguousarray(wkT[:, f * P:(f + 1) * P]),
            "wvT": np.ascontiguousarray(wvT[:, f * P:(f + 1) * P]),
            "fwT": np.ascontiguousarray(fwT[f * P:(f + 1) * P, :]),
            "ident": ident,
        }
        if has_qb:
            m["bq"] = np.ascontiguousarray(bq[f * P:(f + 1) * P])
        if has_kb:
            m["bk"] = np.ascontiguousarray(bk[f * P:(f + 1) * P])
        if has_vb:
            m["bvr"] = np.ascontiguousarray(
                bv[f * P:(f + 1) * P].reshape(1, P).astype(BF16))
        in_maps.append(m)
    return flags, in_maps


def get_program_and_inputs(in_feats, wq, wk, wv, fc_w, fc_b, ln_w, ln_b):
    global _PROGRAM
    flags, in_maps = _prep_inputs(in_feats, wq, wk, wv, fc_w, fc_b, ln_w, ln_b)
    if _PROGRAM is None or _PROGRAM[0] != flags:
        _PROGRAM = (flags, _build_program(*flags))
    return _PROGRAM[1], in_maps


def gather_output(res, in_feats, fc_b):
    halves = []
    for h in range(2):
        acc = res.results[h * NPAIRS]["out"].astype(np.float32).copy()
        for f in range(1, NPAIRS):
            acc += res.results[h * NPAIRS + f]["out"].astype(np.float32)
        halves.append(acc)
    out = np.concatenate(halves, axis=0)
    out += np.asarray(in_feats).astype(np.float32)
    out += np.asarray(fc_b).astype(np.float32)[None, :]
    return np.ascontiguousarray(out)


def kernel(in_feats, wq, wk, wv, fc_w, fc_b, ln_w, ln_b):
    in_feats = np.asarray(in_feats)
    fc_b = np.asarray(fc_b)
    nc, in_maps = get_program_and_inputs(
        in_feats, np.asarray(wq), np.asarray(wk), np.asarray(wv),
        np.asarray(fc_w), fc_b, np.asarray(ln_w), np.asarray(ln_b))
    from concourse.bass_utils import run_bass_kernel_spmd
    res = run_bass_kernel_spmd(nc, in_maps, list(range(NCORES)))
    return gather_output(res, in_feats, fc_b)


# revision 24
# speedup vs baseline: 1.7600x; 1.7600x over previous
"""GAT layer (LayerNorm -> QKV -> full 8-head attention with leaky_relu-before-
softmax -> out-proj -> skip) on 8 Trainium2 NeuronCores.

Sharding: (head-pair, q-half).  Core c handles heads (2f, 2f+1) with f = c % 4
and query rows [h*2048, (h+1)*2048) with h = c // 4 (the host rolls x so each
core's q rows sit at [0, 2048)).  Each core projects k/v for only its two
heads over all 4096 nodes and returns the *partial* fc output for its q-half
(the contribution of its 2 heads).  The host sums the 4 partials per q-half
and adds the skip connection + fc bias -- a pure linear unshard of the
partial-sum sharding.

Per-core pipeline:
  prologue: stream x (bf16), LayerNorm stats (bn_stats/bn_aggr ->
            rstd = exp(-0.5*ln(var+eps)) so only the natural_log_exp table
            set is ever loaded), normalize on GpSimd (bf16), PE-transpose to
            xT for the q-half, project qT and kT/v for kv banks 0-1.
  bank loop (4 q-banks of 512 q) x 32 kv chunks, sp pool depth 3 (6 PSUM
  banks; fc / JIT projections / transposes borrow the same pool):
      scores: two K=64 matmuls row-tiled into PE halves (tile_position (0,0)
              / (64,0), run concurrently) -> sp [128 kv, 2*512] f32 PSUM.
      leaky:  t = 4*relu(s) (DVE tensor_scalar, a few chunks on ACT to
              balance) -> PE identity-accumulate (m = s + 4 relu(s)
              = 5*leaky(s)) -> ACT exp(0.2*m) from PSUM.
      AV:     p.T @ [v|1] per head, M=65 (softmax denominator rides the
              matmul), accumulated over the 32 chunks in PSUM.
    per bank: softmax divide (reciprocal in [128, 8] partition-major layout
    via a DRAM bounce, broadcast back), fc partial (heads stacked K=128),
    DMA out.
  LN/transpose/projection for kv banks 2-7 are emitted just-in-time inside
  bank 0 so the PE/DVE fill the ACT-bound steady state.
"""

import sys

for _p in ("/opt/trn_rl_repo",):
    if _p not in sys.path:
        sys.path.insert(0, _p)

import numpy as np
import ml_dtypes

B, D, H, DH = 4096, 512, 8, 64
P = 128
NCORES = 8
NPAIRS = 4
QH = B // 2
NT = B // P                 # 32 kv chunks
KC = D // P                 # 4 contraction chunks
NB = B // 512               # 8 node banks
QB = QH // 512              # 4 q banks per core
NG = B // (4 * P)           # 8 LN groups of 4 node tiles
NEG_SLOPE = 0.2
LN_EPS = 1e-5
TEMP = float(np.sqrt(D))

BF16 = ml_dtypes.bfloat16

_PROGRAM = None


def _build_program(has_qb, has_kb, has_vb):
    from contextlib import ExitStack

    import concourse.bass as bass
    import concourse.bacc as bacc
    import concourse.tile as tile
    import concourse.mybir as mybir

    dt = mybir.dt
    AF = mybir.ActivationFunctionType
    OP = mybir.AluOpType

    nc = bacc.Bacc("TRN2", target_bir_lowering=False, debug=False)

    x_d = nc.dram_tensor("x", [B, D], dt.bfloat16, kind="ExternalInput").ap()
    wqT_d = nc.dram_tensor("wqT", [D, P], dt.bfloat16, kind="ExternalInput").ap()
    wkT_d = nc.dram_tensor("wkT", [D, P], dt.bfloat16, kind="ExternalInput").ap()
    wvT_d = nc.dram_tensor("wvT", [D, P], dt.bfloat16, kind="ExternalInput").ap()
    fwT_d = nc.dram_tensor("fwT", [P, D], dt.bfloat16, kind="ExternalInput").ap()
    ident_d = nc.dram_tensor("ident", [P, P], dt.bfloat16, kind="ExternalInput").ap()
    bq_d = bk_d = bvr_d = None
    if has_qb:
        bq_d = nc.dram_tensor("bq", [P], dt.float32, kind="ExternalInput").ap()
    if has_kb:
        bk_d = nc.dram_tensor("bk", [P], dt.float32, kind="ExternalInput").ap()
    if has_vb:
        bvr_d = nc.dram_tensor("bvr", [1, P], dt.bfloat16, kind="ExternalInput").ap()
    out_d = nc.dram_tensor("out", [QH, D], dt.float32, kind="ExternalOutput").ap()
    # softmax denominator bounce buffers (flat, [bank*1024 + head*512 + q])
    dden_d = nc.dram_tensor("dden", [QH * 2], dt.bfloat16).ap()
    drec_d = nc.dram_tensor("drec", [QH * 2], dt.bfloat16).ap()

    with tile.TileContext(nc) as tc, ExitStack() as ctx:
        consts = ctx.enter_context(tc.tile_pool(name="consts", bufs=1))
        persist = ctx.enter_context(tc.tile_pool(name="persist", bufs=1))

        ident_t = consts.tile([P, P], dt.bfloat16, name="ident_t", tag="ident")
        nc.sync.dma_start(out=ident_t[:], in_=ident_d)
        eps_t = consts.tile([P, 1], dt.float32, name="eps_t", tag="eps")
        nc.vector.memset(eps_t[:], LN_EPS)
        wq_t = [consts.tile([P, P], dt.bfloat16, name=f"wq{k}", tag=f"wq{k}") for k in range(KC)]
        wk_t = [consts.tile([P, P], dt.bfloat16, name=f"wk{k}", tag=f"wk{k}") for k in range(KC)]
        wv_t = [consts.tile([P, P], dt.bfloat16, name=f"wv{k}", tag=f"wv{k}") for k in range(KC)]
        fw_t = consts.tile([P, D], dt.bfloat16, name="fw", tag="fw")
        bq_t = bk_t = bvr_t = ones1_t = None
        if has_qb:
            bq_t = consts.tile([P, 1], dt.float32, name="bq_t", tag="bq")
        if has_kb:
            bk_t = consts.tile([P, 1], dt.float32, name="bk_t", tag="bk")
        if has_vb:
            bvr_t = consts.tile([1, P], dt.bfloat16, name="bvr_t", tag="bvr")
            ones1_t = consts.tile([1, P], dt.bfloat16, name="ones1_t", tag="ones1")
            nc.vector.memset(ones1_t[:], 1.0)

        def emit_weight_dmas():
            for k in range(KC):
                nc.sync.dma_start(out=wq_t[k][:], in_=wqT_d[k * P:(k + 1) * P, :])
                nc.sync.dma_start(out=wk_t[k][:], in_=wkT_d[k * P:(k + 1) * P, :])
                nc.sync.dma_start(out=wv_t[k][:], in_=wvT_d[k * P:(k + 1) * P, :])
            nc.sync.dma_start(out=fw_t[:], in_=fwT_d)
            if has_qb:
                nc.sync.dma_start(out=bq_t[:, 0], in_=bq_d)
            if has_kb:
                nc.sync.dma_start(out=bk_t[:, 0], in_=bk_d)
            if has_vb:
                nc.sync.dma_start(out=bvr_t[:], in_=bvr_d)

        # ---- persistent tensors ----
        xT = persist.tile([P, KC, B], dt.bfloat16, name="xT", tag="xT")
        kT = persist.tile([P, B], dt.bfloat16, name="kT", tag="kT")
        qT = persist.tile([P, QH], dt.bfloat16, name="qT", tag="qT")
        # vA[:, c, :]: [128 kv, 130]; 0:64 head0 v, 64 = 1, 65:129 head1 v,
        # 129 = 1
        vA = persist.tile([P, NT, 2 * DH + 2], dt.bfloat16, name="vA", tag="vA")
        # both heads' attention outputs stacked: rows 0:64 head0, 64:128 head1
        aT_t = persist.tile([P, QH], dt.bfloat16, name="aT", tag="aT")
        vap = vA[:]
        for j in range(2):
            col = DH + j * (DH + 1)
            ones_dst = bass.AP(tensor=vap.tensor, offset=vap.offset + col,
                               ap=[list(vap.ap[0]), [2 * DH + 2, NT], [1, 1]])
            nc.gpsimd.memset(ones_dst, 1.0)

        # LN stats: mv_t[:, 4g+j, 0] = mean, [..., 1] = var
        mv_t = persist.tile([P, NG * 4, 2], dt.float32, name="mv", tag="mv")
        rstd_t = persist.tile([P, NG * 4], dt.float32, name="rstd", tag="rstd")
        lnv_t = persist.tile([P, NG * 4], dt.float32, name="lnv", tag="lnv")

        with tc.tile_pool(name="xin", bufs=5) as xpool, \
             tc.tile_pool(name="stats", bufs=8) as spool, \
             tc.tile_pool(name="xh", bufs=3) as hpool, \
             tc.tile_pool(name="sps", bufs=3, space="PSUM") as sps, \
             tc.tile_pool(name="aug_ps", bufs=2, space="PSUM") as augps, \
             tc.tile_pool(name="tt", bufs=3) as tpool, \
             tc.tile_pool(name="pt", bufs=2) as ptpool, \
             tc.tile_pool(name="div", bufs=4) as dpool, \
             tc.tile_pool(name="ot", bufs=2) as opool:

            xg_t = [None] * NG

            def ps_tile():
                return sps.tile([P, 1024], dt.float32, tag="sp", name="sp")

            def emit_stats(g):
                xg = xpool.tile([P, 4, D], dt.bfloat16, tag="xg", name="xg")
                src = bass.AP(tensor=x_d.tensor, offset=x_d.offset + 4 * g * P * D,
                              ap=[[D, P], [P * D, 4], [1, D]])
                nc.sync.dma_start(out=xg[:], in_=src)
                xg_t[g] = xg
                for j in range(4):
                    st6 = spool.tile([P, 6], dt.float32, tag="st6", name="st6")
                    nc.vector.bn_stats(st6[:], xg[:, j, :])
                    nc.vector.bn_aggr(mv_t[:, 4 * g + j, :], st6[:])

            def emit_rstd(g0, g1):
                """rstd = exp(-0.5*ln(var+eps)) for groups [g0, g1)."""
                s = slice(4 * g0, 4 * g1)
                nc.scalar.activation(lnv_t[:, s], mv_t[:, s, 1], AF.Ln,
                                     bias=eps_t[:, 0:1])
                nc.scalar.activation(rstd_t[:, s], lnv_t[:, s], AF.Exp,
                                     scale=-0.5)

            def emit_norm_xpose(g):
                xg = xg_t[g]
                for j in range(4):
                    xh = hpool.tile([P, D], dt.bfloat16, tag="xh", name="xh")
                    nc.gpsimd.tensor_scalar(
                        out=xh[:], in0=xg[:, j, :],
                        scalar1=mv_t[:, 4 * g + j, 0:1],
                        scalar2=rstd_t[:, 4 * g + j:4 * g + j + 1],
                        op0=OP.subtract, op1=OP.mult)
                    tpf = ps_tile()
                    tp = tpf[:].bitcast(dt.bfloat16)
                    for f in range(KC):
                        nc.tensor.transpose(
                            tp[:, f * P:(f + 1) * P],
                            xh[:, f * P:(f + 1) * P],
                            ident_t[:],
                        )
                    n0 = (4 * g + j) * P
                    xap = xT[:]
                    dst = bass.AP(tensor=xap.tensor, offset=xap.offset + n0,
                                  ap=[list(xap.ap[0]), [B, KC], [1, P]])
                    nc.vector.tensor_copy(out=dst, in_=tp[:, 0:D])

            def emit_kproj(nb):
                kp = ps_tile()
                for k in range(KC):
                    nc.tensor.matmul(
                        kp[:, 0:512], lhsT=wk_t[k][:],
                        rhs=xT[:, k, nb * 512:(nb + 1) * 512],
                        start=(k == 0), stop=(k == KC - 1))
                if has_kb:
                    nc.vector.tensor_scalar(
                        out=kT[:, nb * 512:(nb + 1) * 512], in0=kp[:, 0:512],
                        scalar1=bk_t[:, 0:1], scalar2=0.0,
                        op0=OP.add, op1=OP.bypass)
                else:
                    nc.vector.tensor_copy(out=kT[:, nb * 512:(nb + 1) * 512],
                                          in_=kp[:, 0:512])

            def emit_vproj(nb):
                vp = ps_tile()
                for blk in range(4):
                    c = nb * 4 + blk
                    for k in range(KC):
                        nc.tensor.matmul(
                            vp[:, blk * P:(blk + 1) * P],
                            lhsT=xT[:, k, c * P:(c + 1) * P],
                            rhs=wv_t[k][:],
                            start=(k == 0), stop=(k == KC - 1 and not has_vb))
                    if has_vb:
                        nc.tensor.matmul(
                            vp[:, blk * P:(blk + 1) * P],
                            lhsT=ones1_t[0:1, :], rhs=bvr_t[0:1, :],
                            start=False, stop=True)
                # one cast per node bank: [128, 4, 2, 64] -> vA[:, 4nb:4nb+4,
                # {0:64, 65:129}]
                dst = bass.AP(
                    tensor=vap.tensor, offset=vap.offset + nb * 4 * (2 * DH + 2),
                    ap=[list(vap.ap[0]), [2 * DH + 2, 4], [DH + 1, 2], [1, DH]])
                pap = vp[:]
                src = bass.AP(
                    tensor=pap.tensor, offset=pap.offset,
                    ap=[list(pap.ap[0]), [P, 4], [DH, 2], [1, DH]])
                nc.vector.tensor_copy(out=dst, in_=src)

            def emit_qproj(qb):
                qp = ps_tile()
                for k in range(KC):
                    nc.tensor.matmul(
                        qp[:, 0:512], lhsT=wq_t[k][:],
                        rhs=xT[:, k, qb * 512:(qb + 1) * 512],
                        start=(k == 0), stop=(k == KC - 1))
                if has_qb:
                    nc.vector.tensor_scalar(
                        out=qT[:, qb * 512:(qb + 1) * 512], in0=qp[:, 0:512],
                        scalar1=bq_t[:, 0:1], scalar2=0.0,
                        op0=OP.add, op1=OP.bypass)
                else:
                    nc.vector.tensor_copy(out=qT[:, qb * 512:(qb + 1) * 512],
                                          in_=qp[:, 0:512])

            # ---------- prologue: q-half (groups 0-3) + kv banks 0-1 ----------
            emit_stats(0)
            emit_weight_dmas()
            for g in range(1, 4):
                emit_stats(g)
            emit_rstd(0, 4)
            for g in range(4):
                emit_norm_xpose(g)
            for qb in range(QB):
                emit_qproj(qb)
            for nb in range(2):
                emit_kproj(nb)
                emit_vproj(nb)

            # ---------- attention ----------
            GR = 4

            def fc_bank(qb):
                for blk in range(4):
                    q0 = qb * 512 + blk * P
                    fpt = ps_tile()
                    fp = fpt[:, 0:512]
                    nc.tensor.matmul(fp, lhsT=aT_t[:, q0:q0 + P],
                                     rhs=fw_t[:], start=True, stop=True)
                    ot = opool.tile([P, D], dt.float32, tag="ot", name="ot")
                    if blk % 2 == 0:
                        nc.vector.tensor_copy(out=ot[:], in_=fp)
                    else:
                        nc.scalar.copy(ot[:], fp)
                    nc.sync.dma_start(out=out_d[q0:q0 + P, :], in_=ot[:])

            # JIT work inside bank 0, keyed by chunk index
            jit = {0: lambda: emit_stats(4),
                   2: lambda: emit_stats(5),
                   3: lambda: (emit_kproj(2), emit_vproj(2)),
                   4: lambda: emit_stats(6),
                   6: lambda: (emit_stats(7), emit_rstd(4, 8)),
                   8: lambda: (emit_kproj(3), emit_vproj(3)),
                   10: lambda: emit_norm_xpose(4),
                   13: lambda: (emit_kproj(4), emit_vproj(4)),
                   16: lambda: emit_norm_xpose(5),
                   18: lambda: (emit_kproj(5), emit_vproj(5)),
                   20: lambda: emit_norm_xpose(6),
                   22: lambda: (emit_kproj(6), emit_vproj(6)),
                   24: lambda: emit_norm_xpose(7),
                   26: lambda: (emit_kproj(7), emit_vproj(7))}

            for qb in range(QB):
                augA = augps.tile([DH + 1, 512], dt.float32, tag="aug")
                augB = augps.tile([DH + 1, 512], dt.float32, tag="aug")
                pt_g = None
                for c in range(NT):
                    if qb == 0 and c in jit:
                        jit[c]()
                    # relu on ACT for a few chunks to balance DVE/ACT load
                    act_path = (c % 4 == 3) if qb == 0 else (c % 16 == 15)
                    if c % GR == 0:
                        pt_g = ptpool.tile([P, GR * 1024], dt.bfloat16, tag="pt",
                                           name="pt")
                    r = c % GR
                    sp = ps_tile()
                    nc.tensor.matmul(
                        sp[:, 0:512],
                        lhsT=kT[0:DH, c * P:(c + 1) * P],
                        rhs=qT[0:DH, qb * 512:(qb + 1) * 512],
                        start=True, stop=True, tile_position=(0, 0))
                    nc.tensor.matmul(
                        sp[:, 512:1024],
                        lhsT=kT[DH:2 * DH, c * P:(c + 1) * P],
                        rhs=qT[DH:2 * DH, qb * 512:(qb + 1) * 512],
                        start=True, stop=True, tile_position=(64, 0))
                    tt = tpool.tile([P, 1024], dt.bfloat16, tag="tt", name="tt")
                    if act_path:
                        nc.scalar.activation(tt[:], sp[:], AF.Relu, scale=4.0)
                    else:
                        nc.vector.tensor_scalar(
                            out=tt[:], in0=sp[:], scalar1=0.0, scalar2=4.0,
                            op0=OP.max, op1=OP.mult)
                    nc.tensor.matmul(
                        sp[:, 0:512], lhsT=ident_t[:], rhs=tt[:, 0:512],
                        start=False, stop=True, skip_group_check=True)
                    nc.tensor.matmul(
                        sp[:, 512:1024], lhsT=ident_t[:], rhs=tt[:, 512:1024],
                        start=False, stop=True, skip_group_check=True)
                    nc.scalar.activation(
                        pt_g[:, r * 1024:(r + 1) * 1024], sp[:],
                        AF.Exp, scale=NEG_SLOPE)
                    nc.tensor.matmul(
                        augA[:], lhsT=vA[:, c, 0:DH + 1],
                        rhs=pt_g[:, r * 1024:r * 1024 + 512],
                        start=(c == 0), stop=(c == NT - 1))
                    nc.tensor.matmul(
                        augB[:], lhsT=vA[:, c, DH + 1:2 * DH + 2],
                        rhs=pt_g[:, r * 1024 + 512:(r + 1) * 1024],
                        start=(c == 0), stop=(c == NT - 1))

                # ---- softmax divide ----
                aug_sb = dpool.tile([DH + 1, 2, 512], dt.bfloat16, tag="augsb",
                                    name="augsb")
                nc.vector.tensor_copy(out=aug_sb[:, 0, :], in_=augA[:])
                nc.vector.tensor_copy(out=aug_sb[:, 1, :], in_=augB[:])
                for j in range(2):
                    nc.sync.dma_start(
                        out=dden_d[qb * 1024 + j * 512:qb * 1024 + (j + 1) * 512],
                        in_=aug_sb[DH:DH + 1, j, :])
                den8 = dpool.tile([P, 8], dt.bfloat16, tag="den8", name="den8")
                src = dden_d[qb * 1024:(qb + 1) * 1024]
                nc.sync.dma_start(
                    out=den8[:],
                    in_=bass.AP(tensor=src.tensor, offset=src.offset,
                                ap=[[8, P], [1, 8]]))
                rec8 = dpool.tile([P, 8], dt.bfloat16, tag="rec8", name="rec8")
                with nc.allow_low_precision(reason="softmax 1/den in bf16 is fine at 2e-2 tol"):
                    nc.vector.reciprocal(rec8[:], den8[:])
                dst = drec_d[qb * 1024:(qb + 1) * 1024]
                nc.sync.dma_start(
                    out=bass.AP(tensor=dst.tensor, offset=dst.offset,
                                ap=[[8, P], [1, 8]]),
                    in_=rec8[:])
                for j in range(2):
                    rb = dpool.tile([DH, 512], dt.bfloat16, tag="rb", name="rb")
                    src = drec_d[qb * 1024 + j * 512:qb * 1024 + (j + 1) * 512]
                    bcast = bass.AP(tensor=src.tensor, offset=src.offset,
                                    ap=[[0, DH], [1, 512]])
                    nc.sync.dma_start(out=rb[:], in_=bcast)
                    nc.vector.tensor_mul(
                        out=aT_t[j * DH:(j + 1) * DH, qb * 512:(qb + 1) * 512],
                        in0=aug_sb[0:DH, j, :], in1=rb[:])
                fc_bank(qb)

    nc.compile()
    return nc


def _prep_inputs(in_feats, wq, wk, wv, fc_w, fc_b, ln_w, ln_b):
    ln_w = ln_w.astype(np.float32)
    ln_b = ln_b.astype(np.float32)
    wq_f = (wq.astype(np.float32) * ln_w[None, :]) / TEMP
    wk_f = wk.astype(np.float32) * ln_w[None, :]
    wv_f = wv.astype(np.float32) * ln_w[None, :]
    bq = (wq.astype(np.float32) @ ln_b) / TEMP
    bk = wk.astype(np.float32) @ ln_b
    bv = wv.astype(np.float32) @ ln_b
    has_qb = bool(np.any(bq != 0))
    has_kb = bool(np.any(bk != 0))
    has_vb = bool(np.any(bv != 0))
    x_bf = np.ascontiguousarray(in_feats.astype(np.float32)).astype(BF16)
    wqT = np.ascontiguousarray(wq_f.T).astype(BF16)
    wkT = np.ascontiguousarray(wk_f.T).astype(BF16)
    wvT = np.ascontiguousarray(wv_f.T).astype(BF16)
    fwT = np.ascontiguousarray(fc_w.astype(np.float32).T).astype(BF16)
    ident = np.eye(P, dtype=np.float32).astype(BF16)
    flags = (has_qb, has_kb, has_vb)
    x_halves = [x_bf, np.ascontiguousarray(np.roll(x_bf, -QH, axis=0))]
    in_maps = []
    for c in range(NCORES):
        f = c % NPAIRS
        h = c // NPAIRS
        m = {
            "x": x_halves[h],
            "wqT": np.ascontiguousarray(wqT[:, f * P:(f + 1) * P]),
            "wkT": np.ascontiguousarray(wkT[:, f * P:(f + 1) * P]),
            "wvT": np.ascontiguousarray(wvT[:, f * P:(f + 1) * P]),
            "fwT": np.ascontiguousarray(fwT[f * P:(f + 1) * P, :]),
            "ident": ident,
        }
        if has_qb:
            m["bq"] = np.ascontiguousarray(bq[f * P:(f + 1) * P])
        if has_kb:
            m["bk"] = np.ascontiguousarray(bk[f * P:(f + 1) * P])
        if has_vb:
            m["bvr"] = np.ascontiguousarray(
                bv[f * P:(f + 1) * P].reshape(1, P).astype(BF16))
        in_maps.append(m)
    return flags, in_maps


def get_program_and_inputs(in_feats, wq, wk, wv, fc_w, fc_b, ln_w, ln_b):
    global _PROGRAM
    flags, in_maps = _prep_inputs(in_feats, wq, wk, wv, fc_w, fc_b, ln_w, ln_b)
    if _PROGRAM is None or _PROGRAM[0] != flags:
        _PROGRAM = (flags, _build_program(*flags))
    return _PROGRAM[1], in_maps


def gather_output(res, in_feats, fc_b):
    halves = []
    for h in range(2):
        acc = res.results[h * NPAIRS]["out"].astype(np.float32).copy()
        for f in range(1, NPAIRS):
            acc += res.results[h * NPAIRS + f]["out"].astype(np.float32)
        halves.append(acc)
    out = np.concatenate(halves, axis=0)
    out += np.asarray(in_feats).astype(np.float32)
    out += np.asarray(fc_b).astype(np.float32)[None, :]
    return np.ascontiguousarray(out)


def kernel(in_feats, wq, wk, wv, fc_w, fc_b, ln_w, ln_b):
    in_feats = np.asarray(in_feats)
    fc_b = np.asarray(fc_b)
    nc, in_maps = get_program_and_inputs(
        in_feats, np.asarray(wq), np.asarray(wk), np.asarray(wv),
        np.asarray(fc_w), fc_b, np.asarray(ln_w), np.asarray(ln_b))
    from concourse.bass_utils import run_bass_kernel_spmd
    res = run_bass_kernel_spmd(nc, in_maps, list(range(NCORES)))
    return gather_output(res, in_feats, fc_b)


# revision 26
# speedup vs baseline: 2.1885x; 1.2435x over previous
"""GAT layer (LayerNorm -> QKV -> full 8-head attention with leaky_relu-before-
softmax -> out-proj -> skip) on 8 Trainium2 NeuronCores.

Sharding: (head-pair, q-half).  Core c handles heads (2f, 2f+1) with f = c % 4
and query rows [h*2048, (h+1)*2048) with h = c // 4 (the host rolls x so each
core's q rows sit at [0, 2048)).  Each core projects k/v for only its two
heads over all 4096 nodes and returns the *partial* fc output for its q-half
(the contribution of its 2 heads).  The host sums the 4 partials per q-half
and adds the skip connection + fc bias -- a pure linear unshard of the
partial-sum sharding.

Per-core pipeline:
  prologue: stream x (bf16), LayerNorm stats (bn_stats/bn_aggr ->
            rstd = exp(-0.5*ln(var+eps)) so only the natural_log_exp table
            set is ever loaded), normalize on GpSimd (bf16), PE-transpose to
            xT for the q-half, project qT and kT/v for kv banks 0-1.
  bank loop (4 q-banks of 512 q) x 32 kv chunks, sp pool depth 3 (6 PSUM
  banks; fc / JIT projections / transposes borrow the same pool):
      scores: two K=64 matmuls row-tiled into PE halves (tile_position (0,0)
              / (64,0), run concurrently) -> sp [128 kv, 2*512] f32 PSUM.
      leaky:  t = 4*relu(s) (DVE tensor_scalar, a few chunks on ACT to
              balance) -> PE identity-accumulate (m = s + 4 relu(s)
              = 5*leaky(s)) -> ACT exp(0.2*m) from PSUM.
      AV:     p.T @ [v|1] per head, M=65 (softmax denominator rides the
              matmul), accumulated over the 32 chunks in PSUM.
    per bank: softmax divide (reciprocal in [128, 8] partition-major layout
    via a DRAM bounce, broadcast back), fc partial (heads stacked K=128),
    DMA out.
  LN/transpose/projection for kv banks 2-7 are emitted just-in-time inside
  bank 0 so the PE/DVE fill the ACT-bound steady state.
"""

import sys

for _p in ("/opt/trn_rl_repo",):
    if _p not in sys.path:
        sys.path.insert(0, _p)

import numpy as np
import ml_dtypes

B, D, H, DH = 4096, 512, 8, 64
P = 128
NCORES = 8
NPAIRS = 4
QH = B // 2
NT = B // P                 # 32 kv chunks
KC = D // P                 # 4 contraction chunks
NB = B // 512               # 8 node banks
QB = QH // 512              # 4 q banks per core
NG = B // (4 * P)           # 8 LN groups of 4 node tiles
NEG_SLOPE = 0.2
LN_EPS = 1e-5
TEMP = float(np.sqrt(D))

BF16 = ml_dtypes.bfloat16

_PROGRAM = None


def _build_program(has_qb, has_kb, has_vb):
    from contextlib import ExitStack

    import concourse.bass as bass
    import concourse.bacc as bacc
    import concourse.tile as tile
    import concourse.mybir as mybir

    dt = mybir.dt
    AF = mybir.ActivationFunctionType
    OP = mybir.AluOpType

    nc = bacc.Bacc("TRN2", target_bir_lowering=False, debug=False)

    x_d = nc.dram_tensor("x", [B, D], dt.bfloat16, kind="ExternalInput").ap()
    wqT_d = nc.dram_tensor("wqT", [D, P], dt.bfloat16, kind="ExternalInput").ap()
    wkT_d = nc.dram_tensor("wkT", [D, P], dt.bfloat16, kind="ExternalInput").ap()
    wvT_d = nc.dram_tensor("wvT", [D, P], dt.bfloat16, kind="ExternalInput").ap()
    fwT_d = nc.dram_tensor("fwT", [P, D], dt.bfloat16, kind="ExternalInput").ap()
    ident_d = nc.dram_tensor("ident", [P, P], dt.bfloat16, kind="ExternalInput").ap()
    bq_d = bk_d = bvr_d = None
    if has_qb:
        bq_d = nc.dram_tensor("bq", [P], dt.float32, kind="ExternalInput").ap()
    if has_kb:
        bk_d = nc.dram_tensor("bk", [P], dt.float32, kind="ExternalInput").ap()
    if has_vb:
        bvr_d = nc.dram_tensor("bvr", [1, P], dt.bfloat16, kind="ExternalInput").ap()
    out_d = nc.dram_tensor("out", [QH, D], dt.float32, kind="ExternalOutput").ap()
    # softmax denominator bounce buffers (flat, [bank*1024 + head*512 + q])
    dden_d = nc.dram_tensor("dden", [QH * 2], dt.bfloat16).ap()
    drec_d = nc.dram_tensor("drec", [QH * 2], dt.bfloat16).ap()

    with tile.TileContext(nc) as tc, ExitStack() as ctx:
        consts = ctx.enter_context(tc.tile_pool(name="consts", bufs=1))
        persist = ctx.enter_context(tc.tile_pool(name="persist", bufs=1))

        ident_t = consts.tile([P, P], dt.bfloat16, name="ident_t", tag="ident")
        nc.sync.dma_start(out=ident_t[:], in_=ident_d)
        eps_t = consts.tile([P, 1], dt.float32, name="eps_t", tag="eps")
        nc.vector.memset(eps_t[:], LN_EPS)
        wq_t = [consts.tile([P, P], dt.bfloat16, name=f"wq{k}", tag=f"wq{k}") for k in range(KC)]
        wk_t = [consts.tile([P, P], dt.bfloat16, name=f"wk{k}", tag=f"wk{k}") for k in range(KC)]
        wv_t = [consts.tile([P, P], dt.bfloat16, name=f"wv{k}", tag=f"wv{k}") for k in range(KC)]
        fw_t = consts.tile([P, D], dt.bfloat16, name="fw", tag="fw")
        bq_t = bk_t = bvr_t = ones1_t = None
        if has_qb:
            bq_t = consts.tile([P, 1], dt.float32, name="bq_t", tag="bq")
        if has_kb:
            bk_t = consts.tile([P, 1], dt.float32, name="bk_t", tag="bk")
        if has_vb:
            bvr_t = consts.tile([1, P], dt.bfloat16, name="bvr_t", tag="bvr")
            ones1_t = consts.tile([1, P], dt.bfloat16, name="ones1_t", tag="ones1")
            nc.vector.memset(ones1_t[:], 1.0)

        def emit_weight_dmas():
            for k in range(KC):
                nc.sync.dma_start(out=wq_t[k][:], in_=wqT_d[k * P:(k + 1) * P, :])
                nc.sync.dma_start(out=wk_t[k][:], in_=wkT_d[k * P:(k + 1) * P, :])
                nc.sync.dma_start(out=wv_t[k][:], in_=wvT_d[k * P:(k + 1) * P, :])
            nc.sync.dma_start(out=fw_t[:], in_=fwT_d)
            if has_qb:
                nc.sync.dma_start(out=bq_t[:, 0], in_=bq_d)
            if has_kb:
                nc.sync.dma_start(out=bk_t[:, 0], in_=bk_d)
            if has_vb:
                nc.sync.dma_start(out=bvr_t[:], in_=bvr_d)

        # ---- persistent tensors ----
        xT = persist.tile([P, KC, B], dt.bfloat16, name="xT", tag="xT")
        kT = persist.tile([P, B], dt.bfloat16, name="kT", tag="kT")
        qT = persist.tile([P, QH], dt.bfloat16, name="qT", tag="qT")
        # vA[:, c, :]: [128 kv, 130]; 0:64 head0 v, 64 = 1, 65:129 head1 v,
        # 129 = 1
        vA = persist.tile([P, NT, 2 * DH + 2], dt.bfloat16, name="vA", tag="vA")
        # both heads' attention outputs stacked: rows 0:64 head0, 64:128 head1
        aT_t = persist.tile([P, QH], dt.bfloat16, name="aT", tag="aT")
        vap = vA[:]
        for j in range(2):
            col = DH + j * (DH + 1)
            ones_dst = bass.AP(tensor=vap.tensor, offset=vap.offset + col,
                               ap=[list(vap.ap[0]), [2 * DH + 2, NT], [1, 1]])
            nc.gpsimd.memset(ones_dst, 1.0)

        # LN stats: mv_t[:, 4g+j, 0] = mean, [..., 1] = var
        mv_t = persist.tile([P, NG * 4, 2], dt.float32, name="mv", tag="mv")
        rstd_t = persist.tile([P, NG * 4], dt.float32, name="rstd", tag="rstd")
        lnv_t = persist.tile([P, NG * 4], dt.float32, name="lnv", tag="lnv")

        with tc.tile_pool(name="xin", bufs=5) as xpool, \
             tc.tile_pool(name="stats", bufs=8) as spool, \
             tc.tile_pool(name="xh", bufs=3) as hpool, \
             tc.tile_pool(name="sps", bufs=3, space="PSUM") as sps, \
             tc.tile_pool(name="aug_ps", bufs=2, space="PSUM") as augps, \
             tc.tile_pool(name="tt", bufs=3) as tpool, \
             tc.tile_pool(name="pt", bufs=2) as ptpool, \
             tc.tile_pool(name="div", bufs=4) as dpool, \
             tc.tile_pool(name="ot", bufs=2) as opool:

            xg_t = [None] * NG

            def ps_tile():
                return sps.tile([P, 1024], dt.float32, tag="sp", name="sp")

            def emit_stats(g):
                xg = xpool.tile([P, 4, D], dt.bfloat16, tag="xg", name="xg")
                src = bass.AP(tensor=x_d.tensor, offset=x_d.offset + 4 * g * P * D,
                              ap=[[D, P], [P * D, 4], [1, D]])
                nc.sync.dma_start(out=xg[:], in_=src)
                xg_t[g] = xg
                for j in range(4):
                    st6 = spool.tile([P, 6], dt.float32, tag="st6", name="st6")
                    nc.vector.bn_stats(st6[:], xg[:, j, :])
                    nc.vector.bn_aggr(mv_t[:, 4 * g + j, :], st6[:])

            def emit_rstd(g0, g1):
                """rstd = exp(-0.5*ln(var+eps)) for groups [g0, g1)."""
                s = slice(4 * g0, 4 * g1)
                nc.scalar.activation(lnv_t[:, s], mv_t[:, s, 1], AF.Ln,
                                     bias=eps_t[:, 0:1])
                nc.scalar.activation(rstd_t[:, s], lnv_t[:, s], AF.Exp,
                                     scale=-0.5)

            def emit_norm_xpose(g):
                xg = xg_t[g]
                for j in range(4):
                    xh = hpool.tile([P, D], dt.bfloat16, tag="xh", name="xh")
                    nc.vector.tensor_scalar(
                        out=xh[:], in0=xg[:, j, :],
                        scalar1=mv_t[:, 4 * g + j, 0:1],
                        scalar2=rstd_t[:, 4 * g + j:4 * g + j + 1],
                        op0=OP.subtract, op1=OP.mult)
                    tpf = ps_tile()
                    tp = tpf[:].bitcast(dt.bfloat16)
                    for f in range(KC):
                        nc.tensor.transpose(
                            tp[:, f * P:(f + 1) * P],
                            xh[:, f * P:(f + 1) * P],
                            ident_t[:],
                        )
                    n0 = (4 * g + j) * P
                    xap = xT[:]
                    dst = bass.AP(tensor=xap.tensor, offset=xap.offset + n0,
                                  ap=[list(xap.ap[0]), [B, KC], [1, P]])
                    nc.vector.tensor_copy(out=dst, in_=tp[:, 0:D])

            def emit_kproj(nb):
                kp = ps_tile()
                for k in range(KC):
                    nc.tensor.matmul(
                        kp[:, 0:512], lhsT=wk_t[k][:],
                        rhs=xT[:, k, nb * 512:(nb + 1) * 512],
                        start=(k == 0), stop=(k == KC - 1))
                if has_kb:
                    nc.vector.tensor_scalar(
                        out=kT[:, nb * 512:(nb + 1) * 512], in0=kp[:, 0:512],
                        scalar1=bk_t[:, 0:1], scalar2=0.0,
                        op0=OP.add, op1=OP.bypass)
                else:
                    nc.vector.tensor_copy(out=kT[:, nb * 512:(nb + 1) * 512],
                                          in_=kp[:, 0:512])

            def emit_vproj(nb):
                vp = ps_tile()
                for blk in range(4):
                    c = nb * 4 + blk
                    for k in range(KC):
                        nc.tensor.matmul(
                            vp[:, blk * P:(blk + 1) * P],
                            lhsT=xT[:, k, c * P:(c + 1) * P],
                            rhs=wv_t[k][:],
                            start=(k == 0), stop=(k == KC - 1 and not has_vb))
                    if has_vb:
                        nc.tensor.matmul(
                            vp[:, blk * P:(blk + 1) * P],
                            lhsT=ones1_t[0:1, :], rhs=bvr_t[0:1, :],
                            start=False, stop=True)
                # one cast per node bank: [128, 4, 2, 64] -> vA[:, 4nb:4nb+4,
                # {0:64, 65:129}]
                dst = bass.AP(
                    tensor=vap.tensor, offset=vap.offset + nb * 4 * (2 * DH + 2),
                    ap=[list(vap.ap[0]), [2 * DH + 2, 4], [DH + 1, 2], [1, DH]])
                pap = vp[:]
                src = bass.AP(
                    tensor=pap.tensor, offset=pap.offset,
                    ap=[list(pap.ap[0]), [P, 4], [DH, 2], [1, DH]])
                nc.vector.tensor_copy(out=dst, in_=src)

            def emit_qproj(qb):
                qp = ps_tile()
                for k in range(KC):
                    nc.tensor.matmul(
                        qp[:, 0:512], lhsT=wq_t[k][:],
                        rhs=xT[:, k, qb * 512:(qb + 1) * 512],
                        start=(k == 0), stop=(k == KC - 1))
                if has_qb:
                    nc.vector.tensor_scalar(
                        out=qT[:, qb * 512:(qb + 1) * 512], in0=qp[:, 0:512],
                        scalar1=bq_t[:, 0:1], scalar2=0.0,
                        op0=OP.add, op1=OP.bypass)
                else:
                    nc.vector.tensor_copy(out=qT[:, qb * 512:(qb + 1) * 512],
                                          in_=qp[:, 0:512])

            # ---------- prologue: q-half (groups 0-3) + kv banks 0-1 ----------
            emit_stats(0)
            emit_weight_dmas()
            for g in range(1, 4):
                emit_stats(g)
            emit_rstd(0, 4)
            for g in range(4):
                emit_norm_xpose(g)
            for qb in range(QB):
                emit_qproj(qb)
            for nb in range(2):
                emit_kproj(nb)
                emit_vproj(nb)

            # ---------- attention ----------
            GR = 4

            def fc_bank(qb):
                for blk in range(4):
                    q0 = qb * 512 + blk * P
                    fpt = ps_tile()
                    fp = fpt[:, 0:512]
                    nc.tensor.matmul(fp, lhsT=aT_t[:, q0:q0 + P],
                                     rhs=fw_t[:], start=True, stop=True)
                    ot = opool.tile([P, D], dt.float32, tag="ot", name="ot")
                    if blk % 2 == 0:
                        nc.vector.tensor_copy(out=ot[:], in_=fp)
                    else:
                        nc.scalar.copy(ot[:], fp)
                    nc.sync.dma_start(out=out_d[q0:q0 + P, :], in_=ot[:])

            # JIT work inside bank 0, keyed by chunk index
            jit = {0: lambda: emit_stats(4),
                   2: lambda: emit_stats(5),
                   3: lambda: (emit_kproj(2), emit_vproj(2)),
                   4: lambda: emit_stats(6),
                   6: lambda: (emit_stats(7), emit_rstd(4, 8)),
                   8: lambda: (emit_kproj(3), emit_vproj(3)),
                   10: lambda: emit_norm_xpose(4),
                   13: lambda: (emit_kproj(4), emit_vproj(4)),
                   16: lambda: emit_norm_xpose(5),
                   18: lambda: (emit_kproj(5), emit_vproj(5)),
                   20: lambda: emit_norm_xpose(6),
                   22: lambda: (emit_kproj(6), emit_vproj(6)),
                   24: lambda: emit_norm_xpose(7),
                   26: lambda: (emit_kproj(7), emit_vproj(7))}

            for qb in range(QB):
                augA = augps.tile([DH + 1, 512], dt.float32, tag="aug")
                augB = augps.tile([DH + 1, 512], dt.float32, tag="aug")
                pt_g = None
                for c in range(NT):
                    if qb == 0 and c in jit:
                        jit[c]()
                    # relu on ACT for a few chunks to balance DVE/ACT load
                    act_path = (c % 4 == 3)
                    if c % GR == 0:
                        pt_g = ptpool.tile([P, GR * 1024], dt.bfloat16, tag="pt",
                                           name="pt")
                    r = c % GR
                    sp = ps_tile()
                    nc.tensor.matmul(
                        sp[:, 0:512],
                        lhsT=kT[0:DH, c * P:(c + 1) * P],
                        rhs=qT[0:DH, qb * 512:(qb + 1) * 512],
                        start=True, stop=True, tile_position=(0, 0))
                    nc.tensor.matmul(
                        sp[:, 512:1024],
                        lhsT=kT[DH:2 * DH, c * P:(c + 1) * P],
                        rhs=qT[DH:2 * DH, qb * 512:(qb + 1) * 512],
                        start=True, stop=True, tile_position=(64, 0))
                    tt = tpool.tile([P, 1024], dt.bfloat16, tag="tt", name="tt")
                    if act_path:
                        nc.scalar.activation(tt[:], sp[:], AF.Relu, scale=4.0)
                    else:
                        nc.vector.tensor_scalar(
                            out=tt[:], in0=sp[:], scalar1=0.0, scalar2=4.0,
                            op0=OP.max, op1=OP.mult)
                    nc.tensor.matmul(
                        sp[:, 0:512], lhsT=ident_t[:], rhs=tt[:, 0:512],
                        start=False, stop=True, skip_group_check=True)
                    nc.tensor.matmul(
                        sp[:, 512:1024], lhsT=ident_t[:], rhs=tt[:, 512:1024],
                        start=False, stop=True, skip_group_check=True)
                    nc.scalar.activation(
                        pt_g[:, r * 1024:(r + 1) * 1024], sp[:],
                        AF.Exp, scale=NEG_SLOPE)
                    nc.tensor.matmul(
                        augA[:], lhsT=vA[:, c, 0:DH + 1],
                        rhs=pt_g[:, r * 1024:r * 1024 + 512],
                        start=(c == 0), stop=(c == NT - 1))
                    nc.tensor.matmul(
                        augB[:], lhsT=vA[:, c, DH + 1:2 * DH + 2],
                        rhs=pt_g[:, r * 1024 + 512:(r + 1) * 1024],
                        start=(c == 0), stop=(c == NT - 1))

                # ---- softmax divide ----
                aug_sb = dpool.tile([DH + 1, 2, 512], dt.bfloat16, tag="augsb",
                                    name="augsb")
                nc.vector.tensor_copy(out=aug_sb[:, 0, :], in_=augA[:])
                nc.vector.tensor_copy(out=aug_sb[:, 1, :], in_=augB[:])
                for j in range(2):
                    nc.sync.dma_start(
                        out=dden_d[qb * 1024 + j * 512:qb * 1024 + (j + 1) * 512],
                        in_=aug_sb[DH:DH + 1, j, :])
                den8 = dpool.tile([P, 8], dt.bfloat16, tag="den8", name="den8")
                src = dden_d[qb * 1024:(qb + 1) * 1024]
                nc.sync.dma_start(
                    out=den8[:],
                    in_=bass.AP(tensor=src.tensor, offset=src.offset,
                                ap=[[8, P], [1, 8]]))
                rec8 = dpool.tile([P, 8], dt.bfloat16, tag="rec8", name="rec8")
                with nc.allow_low_precision(reason="softmax 1/den in bf16 is fine at 2e-2 tol"):
                    nc.vector.reciprocal(rec8[:], den8[:])
                dst = drec_d[qb * 1024:(qb + 1) * 1024]
                nc.sync.dma_start(
                    out=bass.AP(tensor=dst.tensor, offset=dst.offset,
                                ap=[[8, P], [1, 8]]),
                    in_=rec8[:])
                for j in range(2):
                    rb = dpool.tile([DH, 512], dt.bfloat16, tag="rb", name="rb")
                    src = drec_d[qb * 1024 + j * 512:qb * 1024 + (j + 1) * 512]
                    bcast = bass.AP(tensor=src.tensor, offset=src.offset,
                                    ap=[[0, DH], [1, 512]])
                    nc.sync.dma_start(out=rb[:], in_=bcast)
                    nc.vector.tensor_mul(
                        out=aT_t[j * DH:(j + 1) * DH, qb * 512:(qb + 1) * 512],
                        in0=aug_sb[0:DH, j, :], in1=rb[:])
                fc_bank(qb)

    nc.compile()
    return nc


def _prep_inputs(in_feats, wq, wk, wv, fc_w, fc_b, ln_w, ln_b):
    ln_w = ln_w.astype(np.float32)
    ln_b = ln_b.astype(np.float32)
    wq_f = (wq.astype(np.float32) * ln_w[None, :]) / TEMP
    wk_f = wk.astype(np.float32) * ln_w[None, :]
    wv_f = wv.astype(np.float32) * ln_w[None, :]
    bq = (wq.astype(np.float32) @ ln_b) / TEMP
    bk = wk.astype(np.float32) @ ln_b
    bv = wv.astype(np.float32) @ ln_b
    has_qb = bool(np.any(bq != 0))
    has_kb = bool(np.any(bk != 0))
    has_vb = bool(np.any(bv != 0))
    x_bf = np.ascontiguousarray(in_feats.astype(np.float32)).astype(BF16)
    wqT = np.ascontiguousarray(wq_f.T).astype(BF16)
    wkT = np.ascontiguousarray(wk_f.T).astype(BF16)
    wvT = np.ascontiguousarray(wv_f.T).astype(BF16)
    fwT = np.ascontiguousarray(fc_w.astype(np.float32).T).astype(BF16)
    ident = np.eye(P, dtype=np.float32).astype(BF16)
    flags = (has_qb, has_kb, has_vb)
    x_halves = [x_bf, np.ascontiguousarray(np.roll(x_bf, -QH, axis=0))]
    in_maps = []
    for c in range(NCORES):
        f = c % NPAIRS
        h = c // NPAIRS
        m = {
            "x": x_halves[h],
            "wqT": np.ascontiguousarray(wqT[:, f * P:(f + 1) * P]),
            "wkT": np.ascontiguousarray(wkT[:, f * P:(f + 1) * P]),
            "wvT": np.ascontiguousarray(wvT[:, f * P:(f + 1) * P]),
            "fwT": np.ascontiguousarray(fwT[f * P:(f + 1) * P, :]),
            "ident": ident,
        }
        if has_qb:
            m["bq"] = np.ascontiguousarray(bq[f * P:(f + 1) * P])
        if has_kb:
            m["bk"] = np.ascontiguousarray(bk[f * P:(f + 1) * P])
        if has_vb:
            m["bvr"] = np.ascontiguousarray(
                bv[f * P:(f + 1) * P].reshape(1, P).astype(BF16))
        in_maps.append(m)
    return flags, in_maps


def get_program_and_inputs(in_feats, wq, wk, wv, fc_w, fc_b, ln_w, ln_b):
    global _PROGRAM
    flags, in_maps = _prep_inputs(in_feats, wq, wk, wv, fc_w, fc_b, ln_w, ln_b)
    if _PROGRAM is None or _PROGRAM[0] != flags:
        _PROGRAM = (flags, _build_program(*flags))
    return _PROGRAM[1], in_maps


def gather_output(res, in_feats, fc_b):
    halves = []
    for h in range(2):
        acc = res.results[h * NPAIRS]["out"].astype(np.float32).copy()
        for f in range(1, NPAIRS):
            acc += res.results[h * NPAIRS + f]["out"].astype(np.float32)
        halves.append(acc)
    out = np.concatenate(halves, axis=0)
    out += np.asarray(in_feats).astype(np.float32)
    out += np.asarray(fc_b).astype(np.float32)[None, :]
    return np.ascontiguousarray(out)


def kernel(in_feats, wq, wk, wv, fc_w, fc_b, ln_w, ln_b):
    in_feats = np.asarray(in_feats)
    fc_b = np.asarray(fc_b)
    nc, in_maps = get_program_and_inputs(
        in_feats, np.asarray(wq), np.asarray(wk), np.asarray(wv),
        np.asarray(fc_w), fc_b, np.asarray(ln_w), np.asarray(ln_b))
    from concourse.bass_utils import run_bass_kernel_spmd
    res = run_bass_kernel_spmd(nc, in_maps, list(range(NCORES)))
    return gather_output(res, in_feats, fc_b)


# revision 29
# speedup vs baseline: 2.5199x; 1.1514x over previous
"""GAT layer (LayerNorm -> QKV -> full 8-head attention with leaky_relu-before-
softmax -> out-proj -> skip) on 8 Trainium2 NeuronCores.

Sharding: (head-pair, q-half).  Core c handles heads (2f, 2f+1) with f = c % 4
and query rows [h*2048, (h+1)*2048) with h = c // 4 (the host rolls x so each
core's q rows sit at [0, 2048)).  Each core projects k/v for only its two
heads over all 4096 nodes and returns the *partial* fc output for its q-half
(the contribution of its 2 heads).  The host sums the 4 partials per q-half
and adds the skip connection + fc bias -- a pure linear unshard of the
partial-sum sharding.

Per-core pipeline:
  prologue: stream x (bf16), LayerNorm stats (bn_stats/bn_aggr ->
            rstd = exp(-0.5*ln(var+eps)) so only the natural_log_exp table
            set is ever loaded), normalize on GpSimd (bf16), PE-transpose to
            xT for the q-half, project qT and kT/v for kv banks 0-1.
  bank loop (4 q-banks of 512 q) x 32 kv chunks, sp pool depth 3 (6 PSUM
  banks; fc / JIT projections / transposes borrow the same pool):
      scores: two K=64 matmuls row-tiled into PE halves (tile_position (0,0)
              / (64,0), run concurrently) -> sp [128 kv, 2*512] f32 PSUM.
      leaky:  t = 4*relu(s) (DVE tensor_scalar, a few chunks on ACT to
              balance) -> PE identity-accumulate (m = s + 4 relu(s)
              = 5*leaky(s)) -> ACT exp(0.2*m) from PSUM.
      AV:     p.T @ [v|1] per head, M=65 (softmax denominator rides the
              matmul), accumulated over the 32 chunks in PSUM.
    per bank: softmax divide (reciprocal in [128, 8] partition-major layout
    via a DRAM bounce, broadcast back), fc partial (heads stacked K=128),
    DMA out.
  LN/transpose/projection for kv banks 2-7 are emitted just-in-time inside
  bank 0 so the PE/DVE fill the ACT-bound steady state.
"""

import sys

for _p in ("/opt/trn_rl_repo",):
    if _p not in sys.path:
        sys.path.insert(0, _p)

import numpy as np
import ml_dtypes

B, D, H, DH = 4096, 512, 8, 64
P = 128
NCORES = 8
NPAIRS = 4
QH = B // 2
NT = B // P                 # 32 kv chunks
KC = D // P                 # 4 contraction chunks
NB = B // 512               # 8 node banks
QB = QH // 512              # 4 q banks per core
NG = B // (4 * P)           # 8 LN groups of 4 node tiles
NEG_SLOPE = 0.2
LN_EPS = 1e-5
TEMP = float(np.sqrt(D))

BF16 = ml_dtypes.bfloat16

_PROGRAM = None


def _build_program(has_qb, has_kb, has_vb):
    from contextlib import ExitStack

    import concourse.bass as bass
    import concourse.bacc as bacc
    import concourse.tile as tile
    import concourse.mybir as mybir

    dt = mybir.dt
    AF = mybir.ActivationFunctionType
    OP = mybir.AluOpType

    nc = bacc.Bacc("TRN2", target_bir_lowering=False, debug=False)

    x_d = nc.dram_tensor("x", [B, D], dt.bfloat16, kind="ExternalInput").ap()
    wqT_d = nc.dram_tensor("wqT", [D, P], dt.bfloat16, kind="ExternalInput").ap()
    wkT_d = nc.dram_tensor("wkT", [D, P], dt.bfloat16, kind="ExternalInput").ap()
    wvT_d = nc.dram_tensor("wvT", [D, P], dt.bfloat16, kind="ExternalInput").ap()
    fwT_d = nc.dram_tensor("fwT", [P, D], dt.bfloat16, kind="ExternalInput").ap()
    ident_d = nc.dram_tensor("ident", [P, P], dt.bfloat16, kind="ExternalInput").ap()
    bq_d = bk_d = bvr_d = None
    if has_qb:
        bq_d = nc.dram_tensor("bq", [P], dt.float32, kind="ExternalInput").ap()
    if has_kb:
        bk_d = nc.dram_tensor("bk", [P], dt.float32, kind="ExternalInput").ap()
    if has_vb:
        bvr_d = nc.dram_tensor("bvr", [1, P], dt.bfloat16, kind="ExternalInput").ap()
    out_d = nc.dram_tensor("out", [QH, D], dt.float32, kind="ExternalOutput").ap()
    # softmax denominator bounce buffers (flat, [bank*1024 + head*512 + q])
    dden_d = nc.dram_tensor("dden", [QH * 2], dt.bfloat16).ap()
    drec_d = nc.dram_tensor("drec", [QH * 2], dt.bfloat16).ap()

    with tile.TileContext(nc) as tc, ExitStack() as ctx:
        consts = ctx.enter_context(tc.tile_pool(name="consts", bufs=1))
        persist = ctx.enter_context(tc.tile_pool(name="persist", bufs=1))

        ident_t = consts.tile([P, P], dt.bfloat16, name="ident_t", tag="ident")
        nc.sync.dma_start(out=ident_t[:], in_=ident_d)
        eps_t = consts.tile([P, 1], dt.float32, name="eps_t", tag="eps")
        nc.vector.memset(eps_t[:], LN_EPS)
        wq_t = [consts.tile([P, P], dt.bfloat16, name=f"wq{k}", tag=f"wq{k}") for k in range(KC)]
        wk_t = [consts.tile([P, P], dt.bfloat16, name=f"wk{k}", tag=f"wk{k}") for k in range(KC)]
        wv_t = [consts.tile([P, P], dt.bfloat16, name=f"wv{k}", tag=f"wv{k}") for k in range(KC)]
        fw_t = consts.tile([P, D], dt.bfloat16, name="fw", tag="fw")
        bq_t = bk_t = bvr_t = ones1_t = None
        if has_qb:
            bq_t = consts.tile([P, 1], dt.float32, name="bq_t", tag="bq")
        if has_kb:
            bk_t = consts.tile([P, 1], dt.float32, name="bk_t", tag="bk")
        if has_vb:
            bvr_t = consts.tile([1, P], dt.bfloat16, name="bvr_t", tag="bvr")
            ones1_t = consts.tile([1, P], dt.bfloat16, name="ones1_t", tag="ones1")
            nc.vector.memset(ones1_t[:], 1.0)

        def emit_weight_dmas():
            for k in range(KC):
                nc.sync.dma_start(out=wq_t[k][:], in_=wqT_d[k * P:(k + 1) * P, :])
                nc.sync.dma_start(out=wk_t[k][:], in_=wkT_d[k * P:(k + 1) * P, :])
                nc.sync.dma_start(out=wv_t[k][:], in_=wvT_d[k * P:(k + 1) * P, :])
            nc.sync.dma_start(out=fw_t[:], in_=fwT_d)
            if has_qb:
                nc.sync.dma_start(out=bq_t[:, 0], in_=bq_d)
            if has_kb:
                nc.sync.dma_start(out=bk_t[:, 0], in_=bk_d)
            if has_vb:
                nc.sync.dma_start(out=bvr_t[:], in_=bvr_d)

        # ---- persistent tensors ----
        xT = persist.tile([P, KC, B], dt.bfloat16, name="xT", tag="xT")
        kT = persist.tile([P, B], dt.bfloat16, name="kT", tag="kT")
        qT = persist.tile([P, QH], dt.bfloat16, name="qT", tag="qT")
        # vA[:, c, :]: [128 kv, 130]; 0:64 head0 v, 64 = 1, 65:129 head1 v,
        # 129 = 1
        vA = persist.tile([P, NT, 2 * DH + 2], dt.bfloat16, name="vA", tag="vA")
        # both heads' attention outputs stacked: rows 0:64 head0, 64:128 head1
        aT_t = persist.tile([P, QH], dt.bfloat16, name="aT", tag="aT")
        vap = vA[:]
        for j in range(2):
            col = DH + j * (DH + 1)
            ones_dst = bass.AP(tensor=vap.tensor, offset=vap.offset + col,
                               ap=[list(vap.ap[0]), [2 * DH + 2, NT], [1, 1]])
            nc.gpsimd.memset(ones_dst, 1.0)

        # LN stats: mv_t[:, 4g+j, 0] = mean, [..., 1] = var
        mv_t = persist.tile([P, NG * 4, 2], dt.float32, name="mv", tag="mv")
        rstd_t = persist.tile([P, NG * 4], dt.float32, name="rstd", tag="rstd")
        lnv_t = persist.tile([P, NG * 4], dt.float32, name="lnv", tag="lnv")

        with tc.tile_pool(name="xin", bufs=5) as xpool, \
             tc.tile_pool(name="stats", bufs=8) as spool, \
             tc.tile_pool(name="xh", bufs=3) as hpool, \
             tc.tile_pool(name="sps", bufs=3, space="PSUM") as sps, \
             tc.tile_pool(name="aug_ps", bufs=2, space="PSUM") as augps, \
             tc.tile_pool(name="tt", bufs=3) as tpool, \
             tc.tile_pool(name="pt", bufs=2) as ptpool, \
             tc.tile_pool(name="div", bufs=4) as dpool, \
             tc.tile_pool(name="ot", bufs=2) as opool:

            xg_t = [None] * NG

            def ps_tile():
                return sps.tile([P, 1024], dt.float32, tag="sp", name="sp")

            def emit_stats(g):
                xg = xpool.tile([P, 4, D], dt.bfloat16, tag="xg", name="xg")
                src = bass.AP(tensor=x_d.tensor, offset=x_d.offset + 4 * g * P * D,
                              ap=[[D, P], [P * D, 4], [1, D]])
                nc.sync.dma_start(out=xg[:], in_=src)
                xg_t[g] = xg
                for j in range(4):
                    st6 = spool.tile([P, 6], dt.float32, tag="st6", name="st6")
                    nc.vector.bn_stats(st6[:], xg[:, j, :])
                    nc.vector.bn_aggr(mv_t[:, 4 * g + j, :], st6[:])

            def emit_rstd(g0, g1):
                """rstd = exp(-0.5*ln(var+eps)) for groups [g0, g1)."""
                s = slice(4 * g0, 4 * g1)
                nc.scalar.activation(lnv_t[:, s], mv_t[:, s, 1], AF.Ln,
                                     bias=eps_t[:, 0:1])
                nc.scalar.activation(rstd_t[:, s], lnv_t[:, s], AF.Exp,
                                     scale=-0.5)

            def emit_norm_xpose(g):
                xg = xg_t[g]
                for j in range(4):
                    xh = hpool.tile([P, D], dt.bfloat16, tag="xh", name="xh")
                    nc.vector.tensor_scalar(
                        out=xh[:], in0=xg[:, j, :],
                        scalar1=mv_t[:, 4 * g + j, 0:1],
                        scalar2=rstd_t[:, 4 * g + j:4 * g + j + 1],
                        op0=OP.subtract, op1=OP.mult)
                    tpf = ps_tile()
                    tp = tpf[:].bitcast(dt.bfloat16)
                    for f in range(KC):
                        nc.tensor.transpose(
                            tp[:, f * P:(f + 1) * P],
                            xh[:, f * P:(f + 1) * P],
                            ident_t[:],
                        )
                    n0 = (4 * g + j) * P
                    xap = xT[:]
                    dst = bass.AP(tensor=xap.tensor, offset=xap.offset + n0,
                                  ap=[list(xap.ap[0]), [B, KC], [1, P]])
                    nc.vector.tensor_copy(out=dst, in_=tp[:, 0:D])

            def emit_kproj(nb):
                kp = ps_tile()
                for k in range(KC):
                    nc.tensor.matmul(
                        kp[:, 0:512], lhsT=wk_t[k][:],
                        rhs=xT[:, k, nb * 512:(nb + 1) * 512],
                        start=(k == 0), stop=(k == KC - 1))
                if has_kb:
                    nc.vector.tensor_scalar(
                        out=kT[:, nb * 512:(nb + 1) * 512], in0=kp[:, 0:512],
                        scalar1=bk_t[:, 0:1], scalar2=0.0,
                        op0=OP.add, op1=OP.bypass)
                else:
                    nc.vector.tensor_copy(out=kT[:, nb * 512:(nb + 1) * 512],
                                          in_=kp[:, 0:512])

            def emit_vproj(nb):
                vp = ps_tile()
                for blk in range(4):
                    c = nb * 4 + blk
                    for k in range(KC):
                        nc.tensor.matmul(
                            vp[:, blk * P:(blk + 1) * P],
                            lhsT=xT[:, k, c * P:(c + 1) * P],
                            rhs=wv_t[k][:],
                            start=(k == 0), stop=(k == KC - 1 and not has_vb))
                    if has_vb:
                        nc.tensor.matmul(
                            vp[:, blk * P:(blk + 1) * P],
                            lhsT=ones1_t[0:1, :], rhs=bvr_t[0:1, :],
                            start=False, stop=True)
                # one cast per node bank: [128, 4, 2, 64] -> vA[:, 4nb:4nb+4,
                # {0:64, 65:129}]
                dst = bass.AP(
                    tensor=vap.tensor, offset=vap.offset + nb * 4 * (2 * DH + 2),
                    ap=[list(vap.ap[0]), [2 * DH + 2, 4], [DH + 1, 2], [1, DH]])
                pap = vp[:]
                src = bass.AP(
                    tensor=pap.tensor, offset=pap.offset,
                    ap=[list(pap.ap[0]), [P, 4], [DH, 2], [1, DH]])
                nc.vector.tensor_copy(out=dst, in_=src)

            def emit_qproj(qb):
                qp = ps_tile()
                for k in range(KC):
                    nc.tensor.matmul(
                        qp[:, 0:512], lhsT=wq_t[k][:],
                        rhs=xT[:, k, qb * 512:(qb + 1) * 512],
                        start=(k == 0), stop=(k == KC - 1))
                if has_qb:
                    nc.vector.tensor_scalar(
                        out=qT[:, qb * 512:(qb + 1) * 512], in0=qp[:, 0:512],
                        scalar1=bq_t[:, 0:1], scalar2=0.0,
                        op0=OP.add, op1=OP.bypass)
                else:
                    nc.vector.tensor_copy(out=qT[:, qb * 512:(qb + 1) * 512],
                                          in_=qp[:, 0:512])

            # ---------- prologue: q-half (groups 0-3) + kv banks 0-1 ----------
            emit_stats(0)
            emit_weight_dmas()
            for g in range(1, 4):
                emit_stats(g)
            emit_rstd(0, 4)
            for g in range(4):
                emit_norm_xpose(g)
            for qb in range(QB):
                emit_qproj(qb)
            for nb in range(2):
                emit_kproj(nb)
                emit_vproj(nb)

            # ---------- attention ----------
            GR = 4

            def fc_blk(qb, blk):
                q0 = qb * 512 + blk * P
                fpt = ps_tile()
                fp = fpt[:, 0:512]
                nc.tensor.matmul(fp, lhsT=aT_t[:, q0:q0 + P],
                                 rhs=fw_t[:], start=True, stop=True)
                ot = opool.tile([P, D], dt.float32, tag="ot", name="ot")
                if blk % 2 == 0:
                    nc.vector.tensor_copy(out=ot[:], in_=fp)
                else:
                    nc.scalar.copy(ot[:], fp)
                nc.sync.dma_start(out=out_d[q0:q0 + P, :], in_=ot[:])

            def den_recip(qb, aug_sb):
                for j in range(2):
                    nc.sync.dma_start(
                        out=dden_d[qb * 1024 + j * 512:qb * 1024 + (j + 1) * 512],
                        in_=aug_sb[DH:DH + 1, j, :])
                den8 = dpool.tile([P, 8], dt.bfloat16, tag="den8", name="den8")
                src = dden_d[qb * 1024:(qb + 1) * 1024]
                nc.sync.dma_start(
                    out=den8[:],
                    in_=bass.AP(tensor=src.tensor, offset=src.offset,
                                ap=[[8, P], [1, 8]]))
                rec8 = dpool.tile([P, 8], dt.bfloat16, tag="rec8", name="rec8")
                with nc.allow_low_precision(reason="softmax 1/den in bf16 ok at 2e-2 tol"):
                    nc.vector.reciprocal(rec8[:], den8[:])
                dst = drec_d[qb * 1024:(qb + 1) * 1024]
                nc.sync.dma_start(
                    out=bass.AP(tensor=dst.tensor, offset=dst.offset,
                                ap=[[8, P], [1, 8]]),
                    in_=rec8[:])

            def divide_head(qb, aug_sb, j):
                rb = dpool.tile([DH, 512], dt.bfloat16, tag="rb", name="rb")
                src = drec_d[qb * 1024 + j * 512:qb * 1024 + (j + 1) * 512]
                bcast = bass.AP(tensor=src.tensor, offset=src.offset,
                                ap=[[0, DH], [1, 512]])
                nc.sync.dma_start(out=rb[:], in_=bcast)
                nc.vector.tensor_mul(
                    out=aT_t[j * DH:(j + 1) * DH, qb * 512:(qb + 1) * 512],
                    in0=aug_sb[0:DH, j, :], in1=rb[:])

            # JIT work inside bank 0, keyed by chunk index
            jit = {0: lambda: emit_stats(4),
                   2: lambda: emit_stats(5),
                   3: lambda: (emit_kproj(2), emit_vproj(2)),
                   4: lambda: emit_stats(6),
                   6: lambda: (emit_stats(7), emit_rstd(4, 8)),
                   8: lambda: (emit_kproj(3), emit_vproj(3)),
                   10: lambda: emit_norm_xpose(4),
                   13: lambda: (emit_kproj(4), emit_vproj(4)),
                   16: lambda: emit_norm_xpose(5),
                   18: lambda: (emit_kproj(5), emit_vproj(5)),
                   20: lambda: emit_norm_xpose(6),
                   22: lambda: (emit_kproj(6), emit_vproj(6)),
                   24: lambda: emit_norm_xpose(7),
                   26: lambda: (emit_kproj(7), emit_vproj(7))}

            pending = {}
            for qb in range(QB):
                augA = augps.tile([DH + 1, 512], dt.float32, tag="aug")
                augB = augps.tile([DH + 1, 512], dt.float32, tag="aug")
                pt_g = None
                sched = pending
                pending = {}
                for c in range(NT):
                    if qb == 0 and c in jit:
                        jit[c]()
                    if c in sched:
                        sched[c]()
                    # relu on ACT for a few chunks to balance DVE/ACT load
                    act_path = (c % 4 == 3) if qb == 0 else (c % 6 == 5)
                    if c % GR == 0:
                        pt_g = ptpool.tile([P, GR * 1024], dt.bfloat16, tag="pt",
                                           name="pt")
                    r = c % GR
                    sp = ps_tile()
                    nc.tensor.matmul(
                        sp[:, 0:512],
                        lhsT=kT[0:DH, c * P:(c + 1) * P],
                        rhs=qT[0:DH, qb * 512:(qb + 1) * 512],
                        start=True, stop=True, tile_position=(0, 0))
                    nc.tensor.matmul(
                        sp[:, 512:1024],
                        lhsT=kT[DH:2 * DH, c * P:(c + 1) * P],
                        rhs=qT[DH:2 * DH, qb * 512:(qb + 1) * 512],
                        start=True, stop=True, tile_position=(64, 0))
                    tt = tpool.tile([P, 1024], dt.bfloat16, tag="tt", name="tt")
                    if act_path:
                        nc.scalar.activation(tt[:], sp[:], AF.Relu, scale=4.0)
                    else:
                        nc.vector.tensor_scalar(
                            out=tt[:], in0=sp[:], scalar1=0.0, scalar2=4.0,
                            op0=OP.max, op1=OP.mult)
                    nc.tensor.matmul(
                        sp[:, 0:512], lhsT=ident_t[:], rhs=tt[:, 0:512],
                        start=False, stop=True, skip_group_check=True)
                    nc.tensor.matmul(
                        sp[:, 512:1024], lhsT=ident_t[:], rhs=tt[:, 512:1024],
                        start=False, stop=True, skip_group_check=True)
                    nc.scalar.activation(
                        pt_g[:, r * 1024:(r + 1) * 1024], sp[:],
                        AF.Exp, scale=NEG_SLOPE)
                    nc.tensor.matmul(
                        augA[:], lhsT=vA[:, c, 0:DH + 1],
                        rhs=pt_g[:, r * 1024:r * 1024 + 512],
                        start=(c == 0), stop=(c == NT - 1))
                    nc.tensor.matmul(
                        augB[:], lhsT=vA[:, c, DH + 1:2 * DH + 2],
                        rhs=pt_g[:, r * 1024 + 512:(r + 1) * 1024],
                        start=(c == 0), stop=(c == NT - 1))

                # ---- softmax divide: casts now (frees aug for the next
                # bank); the DMA-bounce/reciprocal/fc are deferred into the
                # next bank's chunk stream ----
                aug_sb = dpool.tile([DH + 1, 2, 512], dt.bfloat16, tag="augsb",
                                    name="augsb")
                nc.vector.tensor_copy(out=aug_sb[:, 0, :], in_=augA[:])
                nc.vector.tensor_copy(out=aug_sb[:, 1, :], in_=augB[:])

                def _den(qb=qb, sb=aug_sb):
                    den_recip(qb, sb)

                def _dh0(qb=qb, sb=aug_sb):
                    divide_head(qb, sb, 0)

                def _dh1(qb=qb, sb=aug_sb):
                    divide_head(qb, sb, 1)

                def _fc(qb=qb):
                    return lambda blk: fc_blk(qb, blk)

                if qb < QB - 1:
                    fcf = _fc()
                    pending = {1: _den, 3: _dh0, 5: _dh1,
                               8: lambda f=fcf: f(0), 11: lambda f=fcf: f(1),
                               14: lambda f=fcf: f(2), 17: lambda f=fcf: f(3)}
                else:
                    _den()
                    _dh0()
                    _dh1()
                    for blk in range(4):
                        fc_blk(qb, blk)

    nc.compile()
    return nc


def _prep_inputs(in_feats, wq, wk, wv, fc_w, fc_b, ln_w, ln_b):
    ln_w = ln_w.astype(np.float32)
    ln_b = ln_b.astype(np.float32)
    wq_f = (wq.astype(np.float32) * ln_w[None, :]) / TEMP
    wk_f = wk.astype(np.float32) * ln_w[None, :]
    wv_f = wv.astype(np.float32) * ln_w[None, :]
    bq = (wq.astype(np.float32) @ ln_b) / TEMP
    bk = wk.astype(np.float32) @ ln_b
    bv = wv.astype(np.float32) @ ln_b
    has_qb = bool(np.any(bq != 0))
    has_kb = bool(np.any(bk != 0))
    has_vb = bool(np.any(bv != 0))
    x_bf = np.ascontiguousarray(in_feats.astype(np.float32)).astype(BF16)
    wqT = np.ascontiguousarray(wq_f.T).astype(BF16)
    wkT = np.ascontiguousarray(wk_f.T).astype(BF16)
    wvT = np.ascontiguousarray(wv_f.T).astype(BF16)
    fwT = np.ascontiguousarray(fc_w.astype(np.float32).T).astype(BF16)
    ident = np.eye(P, dtype=np.float32).astype(BF16)
    flags = (has_qb, has_kb, has_vb)
    x_halves = [x_bf, np.ascontiguousarray(np.roll(x_bf, -QH, axis=0))]
    in_maps = []
    for c in range(NCORES):
        f = c % NPAIRS
        h = c // NPAIRS
        m = {
            "x": x_halves[h],
            "wqT": np.ascontiguousarray(wqT[:, f * P:(f + 1) * P]),
            "wkT": np.ascontiguousarray(wkT[:, f * P:(f + 1) * P]),
            "wvT": np.ascontiguousarray(wvT[:, f * P:(f + 1) * P]),
            "fwT": np.ascontiguousarray(fwT[f * P:(f + 1) * P, :]),
            "ident": ident,
        }
        if has_qb:
            m["bq"] = np.ascontiguousarray(bq[f * P:(f + 1) * P])
        if has_kb:
            m["bk"] = np.ascontiguousarray(bk[f * P:(f + 1) * P])
        if has_vb:
            m["bvr"] = np.ascontiguousarray(
                bv[f * P:(f + 1) * P].reshape(1, P).astype(BF16))
        in_maps.append(m)
    return flags, in_maps


def get_program_and_inputs(in_feats, wq, wk, wv, fc_w, fc_b, ln_w, ln_b):
    global _PROGRAM
    flags, in_maps = _prep_inputs(in_feats, wq, wk, wv, fc_w, fc_b, ln_w, ln_b)
    if _PROGRAM is None or _PROGRAM[0] != flags:
        _PROGRAM = (flags, _build_program(*flags))
    return _PROGRAM[1], in_maps


def gather_output(res, in_feats, fc_b):
    halves = []
    for h in range(2):
        acc = res.results[h * NPAIRS]["out"].astype(np.float32).copy()
        for f in range(1, NPAIRS):
            acc += res.results[h * NPAIRS + f]["out"].astype(np.float32)
        halves.append(acc)
    out = np.concatenate(halves, axis=0)
    out += np.asarray(in_feats).astype(np.float32)
    out += np.asarray(fc_b).astype(np.float32)[None, :]
    return np.ascontiguousarray(out)


def kernel(in_feats, wq, wk, wv, fc_w, fc_b, ln_w, ln_b):
    in_feats = np.asarray(in_feats)
    fc_b = np.asarray(fc_b)
    nc, in_maps = get_program_and_inputs(
        in_feats, np.asarray(wq), np.asarray(wk), np.asarray(wv),
        np.asarray(fc_w), fc_b, np.asarray(ln_w), np.asarray(ln_b))
    from concourse.bass_utils import run_bass_kernel_spmd
    res = run_bass_kernel_spmd(nc, in_maps, list(range(NCORES)))
    return gather_output(res, in_feats, fc_b)
